# revision 21
# baseline (speedup 1.0000x reference)
"""LorentzTransformer Trainium2 kernel.

Full inputs in, full output out. Sharding: 8 cores = 2 batches x 4 head
groups (4 heads / 256 channels each). Host pre-transposes x and the weight
shards so every on-chip matmul has its contraction dim on partitions.

Per-core pipeline (all on-chip, fp32 storage):
  QT/KT = W-proj of x (head channels on partitions, seq on free)
  V     = natural-layout proj, augmented with a ones column (softmax denom)
  Qeff  = Q * (1 - 2*alpha*sf*m) / sqrt(dh); sf via PE partition-sum matmuls
  scoresT[k,q] per head -> exp on ACT -> causal via one triangular 0/1 tile
  AV + denom in one PSUM accumulation group; normalize via partition_broadcast
  partial out = A @ Wo_shard.T -> host sums the 4 head-group partials per batch
"""

import numpy as np

from concourse import bacc
import concourse.tile as tile
import concourse.mybir as mybir
from concourse.bass_utils import run_bass_kernel_spmd

B, L, D, H = 2, 1024, 1024, 16
DH = D // H  # 64
ALPHA = 0.25
SCALE = float(np.sqrt(DH))  # 8.0
HPC = 4          # heads per core
DPC = HPC * DH   # 256 channels per core
N_CORES = 8
P = 128
NQC = L // 512   # q chunks of 512
NKT = L // P     # k tiles of 128

FP = mybir.dt.float32
# PE compute dtype: fp16 runs the PE at full rate on the normal datapath
# (the HAM clock gate ignores fp32r matmuls and throttles to 1.2 GHz), gets
# fast-weight-load, and keeps 11 mantissa bits. PSUM accumulation is fp32.
FPC = mybir.dt.float16
NPC = np.float16




def _build_program():
    nc = bacc.Bacc("TRN2", target_bir_lowering=False)

    xT = nc.dram_tensor("xT", [D, L], FPC, kind="ExternalInput")
    wqT = nc.dram_tensor("wqT", [D, DPC], FPC, kind="ExternalInput")
    wkT = nc.dram_tensor("wkT", [D, DPC], FPC, kind="ExternalInput")
    wvT = nc.dram_tensor("wvT", [D, DPC], FPC, kind="ExternalInput")
    woT = nc.dram_tensor("woT", [DPC, D], FPC, kind="ExternalInput")
    normblk = nc.dram_tensor("normblk", [P, 2, 4], FPC, kind="ExternalInput")
    sprime = nc.dram_tensor("sprime", [2, 2, P], FPC, kind="ExternalInput")
    maskT = nc.dram_tensor("maskT", [P, P], FPC, kind="ExternalInput")
    out = nc.dram_tensor("out", [L, D], FP, kind="ExternalOutput")

    with tile.TileContext(nc) as tc:
        with (
            tc.tile_pool(name="persist", bufs=1) as persist,
            tc.tile_pool(name="work", bufs=3) as work,
            tc.tile_pool(name="expp", bufs=6) as expp,
            tc.tile_pool(name="sm", bufs=4) as smp,
            tc.tile_pool(name="ost", bufs=3) as ost,
            tc.tile_pool(name="psA", bufs=2, space="PSUM") as psA,
            tc.tile_pool(name="psS", bufs=3, space="PSUM") as psS,
            tc.tile_pool(name="psV", bufs=3, space="PSUM") as psV,
        ):
            # ---- load inputs ----
            xT_r = xT.rearrange("(o p) l -> p o l", p=P)
            xT_sb = [persist.tile([P, L], FPC, tag=f"xT{k}", name=f"xT{k}") for k in range(D // P)]
            wq_r = wqT.rearrange("(o p) n -> p o n", p=P)
            wq_sb = [persist.tile([P, DPC], FPC, tag=f"wq{k}", name=f"wq{k}") for k in range(D // P)]
            wk_r = wkT.rearrange("(o p) n -> p o n", p=P)
            wk_sb = [persist.tile([P, DPC], FPC, tag=f"wk{k}", name=f"wk{k}") for k in range(D // P)]
            wv_r = wvT.rearrange("(o p) n -> p o n", p=P)
            wv_sb = [persist.tile([P, DPC], FPC, tag=f"wv{k}", name=f"wv{k}") for k in range(D // P)]
            for k in range(D // P):
                nc.sync.dma_start(xT_sb[k][:], xT_r[:, k])
                nc.sync.dma_start(wq_sb[k][:], wq_r[:, k])
            for k in range(D // P):
                nc.sync.dma_start(wk_sb[k][:], wk_r[:, k])
                nc.sync.dma_start(wv_sb[k][:], wv_r[:, k])
            wo_sb = persist.tile([P, DPC // P, D], FPC, tag="wo")
            nc.sync.dma_start(wo_sb[:], woT.rearrange("(o p) n -> p o n", p=P))
            nb_sb = persist.tile([P, 2, 4], FPC, tag="nb")
            nc.sync.dma_start(nb_sb[:], normblk[:])
            sp_sb = persist.tile([2, 2, P], FPC, tag="sp")
            nc.sync.dma_start(sp_sb[:], sprime[:])
            mk_sb = persist.tile([P, P], FPC, tag="mk")
            nc.sync.dma_start(mk_sb[:], maskT[:])

            qT_sb = [persist.tile([P, L], FPC, tag=f"qT{t}", name=f"qT{t}") for t in range(2)]
            kT_sb = [persist.tile([P, L], FPC, tag=f"kT{t}", name=f"kT{t}") for t in range(2)]
            # V' with ones column per (ktile, head)
            v_sb = persist.tile([P, NKT, HPC, DH + 1], FPC, tag="v")
            onecol = persist.tile([P, 1], FP, tag="onecol")
            nc.vector.memset(onecol[:], 1.0)
            nc.vector.tensor_copy(
                v_sb[:, :, :, DH : DH + 1],
                onecol.to_broadcast([P, NKT, HPC, 1]),
            )

            aT_sb = [
                [
                    persist.tile([P, 512], FPC, tag=f"aT{t}_{qc}", name=f"aT{t}_{qc}")
                    for qc in range(NQC)
                ]
                for t in range(2)
            ]

            # ---- QT proj (t-tile at a time) + Lorentz factor, then KT, V ----
            def proj(w_sb, dst, t):
                for qc in range(NQC):
                    ps = psA.tile([P, 512], FP, tag="psA", name="proj")
                    for k in range(D // P):
                        nc.tensor.matmul(
                            ps[:],
                            w_sb[k][:, t * P : (t + 1) * P],
                            xT_sb[k][:, qc * 512 : (qc + 1) * 512],
                            start=(k == 0),
                            stop=(k == D // P - 1),
                        )
                    nc.vector.tensor_copy(dst[t][:, qc * 512 : (qc + 1) * 512], ps[:])

            def lorentz(t):
                # QeffT = QT * (0.125 - 0.0625*sf*m), sf = |Q|/|Qt| per (head,q)
                sq = work.tile([P, L], FPC, tag="sq")
                nc.vector.tensor_mul(sq[:], qT_sb[t][:], qT_sb[t][:])
                sf = work.tile([2, L], FPC, tag="sf")
                for qc in range(NQC):
                    nps = psS.tile([P, 512], FP, tag="psS", name="nps")
                    nc.tensor.matmul(
                        nps[:2, :],
                        nb_sb[:, t, 0:2],
                        sq[:, qc * 512 : (qc + 1) * 512],
                        start=True,
                        stop=True,
                    )
                    nqs = psS.tile([P, 512], FP, tag="psS", name="nqs")
                    nc.tensor.matmul(
                        nqs[:2, :],
                        nb_sb[:, t, 2:4],
                        sq[:, qc * 512 : (qc + 1) * 512],
                        start=True,
                        stop=True,
                    )
                    brcp = smp.tile([2, 512], FP, tag="brcp")
                    nc.vector.reciprocal_approx_fast(brcp[:], nqs[0:2, :])
                    rat = smp.tile([2, 512], FP, tag="rat")
                    nc.vector.tensor_mul(rat[:], nps[0:2, :], brcp[:])
                    nc.scalar.activation(
                        sf[:, qc * 512 : (qc + 1) * 512],
                        rat[:],
                        mybir.ActivationFunctionType.Sqrt,
                    )
                for qc in range(NQC):
                    gps = psS.tile([P, 512], FP, tag="psS", name="gps")
                    nc.tensor.matmul(
                        gps[:],
                        sp_sb[:, t, :],
                        sf[:, qc * 512 : (qc + 1) * 512],
                        start=True,
                        stop=True,
                    )
                    fp_sb = smp.tile([P, 512], FPC, tag="fp")
                    nc.vector.tensor_scalar_add(fp_sb[:], gps[:], 1.0 / SCALE)
                    nc.vector.tensor_mul(
                        qT_sb[t][:, qc * 512 : (qc + 1) * 512],
                        qT_sb[t][:, qc * 512 : (qc + 1) * 512],
                        fp_sb[:],
                    )

            for t in range(2):
                proj(wq_sb, qT_sb, t)
                lorentz(t)
            for t in range(2):
                proj(wk_sb, kT_sb, t)

            # ---- V natural layout: out[l, dv], packed into V' ----
            for lt in range(NKT):
                ps = psA.tile([P, 512], FP, tag="psA", name="vproj")
                for k in range(D // P):
                    nc.tensor.matmul(
                        ps[:, :DPC],
                        xT_sb[k][:, lt * P : (lt + 1) * P],
                        wv_sb[k][:, :],
                        start=(k == 0),
                        stop=(k == D // P - 1),
                    )
                nc.vector.tensor_copy(
                    v_sb[:, lt, :, :DH],
                    ps[:, :DPC].rearrange("p (h d) -> p h d", h=HPC),
                )

            # ---- attention (head pairs row-packed) interleaved with Wo ----
            def attn_group(t, qc):
                avs = [
                    psV.tile([DH + 1, 512], FP, tag="psV", name=f"av{hl}")
                    for hl in range(2)
                ]
                nkt = 4 * qc + 4  # causal: k tiles 0..4qc+3
                for kt in range(nkt):
                    off = max(0, (kt - 4 * qc) * P)  # first visible q col
                    n = 512 - off
                    exs = []
                    for hl in range(2):
                        base = hl * DH
                        sc = psS.tile([P, 512], FP, tag="psS", name=f"sc{hl}")
                        nc.tensor.matmul(
                            sc[:, off:512],
                            kT_sb[t][base : base + DH, kt * P : (kt + 1) * P],
                            qT_sb[t][
                                base : base + DH,
                                qc * 512 + off : (qc + 1) * 512,
                            ],
                            start=True,
                            stop=True,
                            tile_position=(base, 0),
                        )
                        ex = expp.tile([P, 512], FPC, tag="ex", name=f"ex{hl}")
                        nc.scalar.activation(
                            ex[:, off:512],
                            sc[:, off:512],
                            mybir.ActivationFunctionType.Exp,
                        )
                        j = kt - 4 * qc
                        if j >= 0:  # diagonal block gets the triangular mask
                            nc.vector.tensor_mul(
                                ex[:, j * P : (j + 1) * P],
                                ex[:, j * P : (j + 1) * P],
                                mk_sb[:],
                            )
                        exs.append(ex)
                    for hl in range(2):
                        nc.tensor.matmul(
                            avs[hl][:, off:512],
                            v_sb[:, kt, 2 * t + hl, :],
                            exs[hl][:, off:512],
                            start=(kt == 0),
                            stop=(kt == nkt - 1),
                        )
                for hl in range(2):
                    base = hl * DH
                    # free the PSUM bank fast: copy numerator + denom out
                    avr = work.tile([DH, 512], FP, tag="avr", name="avr")
                    nc.vector.tensor_copy(avr[:], avs[hl][:DH, :])
                    den = smp.tile([1, 512], FP, tag="den")
                    nc.scalar.copy(den[:], avs[hl][DH : DH + 1, :])
                    rc = smp.tile([1, 512], FP, tag="rc")
                    nc.vector.reciprocal_approx_fast(rc[:], den[:])
                    bc = smp.tile([DH, 512], FP, tag="bc")
                    nc.gpsimd.partition_broadcast(bc[:], rc[:], channels=DH)
                    nc.vector.tensor_mul(
                        aT_sb[t][qc][base : base + DH, :],
                        avr[:],
                        bc[:],
                    )

            def wo_tile(lt):
                qc = lt // 4
                for jc in range(NQC):
                    ps = psA.tile([P, 512], FP, tag="psA", name="wops")
                    for t2 in range(2):
                        nc.tensor.matmul(
                            ps[:],
                            aT_sb[t2][qc][:, (lt % 4) * P : (lt % 4 + 1) * P],
                            wo_sb[:, t2, jc * 512 : (jc + 1) * 512],
                            start=(t2 == 0),
                            stop=(t2 == 1),
                        )
                    oc = ost.tile([P, 512], FP, tag="oc")
                    nc.scalar.copy(oc[:], ps[:])
                    nc.sync.dma_start(
                        out[lt * P : (lt + 1) * P, jc * 512 : (jc + 1) * 512], oc[:]
                    )

            for qc in range(NQC):
                for t in range(2):
                    attn_group(t, qc)
                for lt in range(4 * qc, 4 * qc + 4):
                    wo_tile(lt)

    nc.compile()
    return nc


_NC = None


def _host_inputs(x, Wq, Wk, Wv, Wo, timelike_mask):
    m_full = np.asarray(timelike_mask).astype(np.float32)
    mt = np.tril(np.ones((P, P), dtype=np.float32)).T.copy()  # maskT[k,q]=1 iff k<=q
    in_maps = []
    for c in range(N_CORES):
        b, g = divmod(c, HPC)
        sl = slice(g * DPC, (g + 1) * DPC)
        m = m_full[sl]  # [256]
        nb = np.zeros((P, 2, 4), dtype=np.float32)
        sp = np.zeros((2, 2, P), dtype=np.float32)
        for t in range(2):
            m_t = m[t * P : (t + 1) * P]
            nb[0:DH, t, 0] = 1.0
            nb[DH:P, t, 1] = 1.0
            nb[0:DH, t, 2] = m_t[0:DH]
            nb[DH:P, t, 3] = m_t[DH:P]
            coef = -2.0 * ALPHA / SCALE  # -0.0625
            sp[0, t, 0:DH] = coef * m_t[0:DH]
            sp[1, t, DH:P] = coef * m_t[DH:P]
        in_maps.append(
            {
                "xT": np.ascontiguousarray(x[b].T).astype(NPC),
                "wqT": np.ascontiguousarray(Wq[sl, :].T).astype(NPC),
                "wkT": np.ascontiguousarray(Wk[sl, :].T).astype(NPC),
                "wvT": np.ascontiguousarray(Wv[sl, :].T).astype(NPC),
                "woT": np.ascontiguousarray(Wo[:, sl].T).astype(NPC),
                "normblk": nb.astype(NPC),
                "sprime": sp.astype(NPC),
                "maskT": mt.astype(NPC),
            }
        )
    return in_maps


def kernel(x, Wq, Wk, Wv, Wo, timelike_mask, attn_mask, _trace=False):
    global _NC
    if _NC is None:
        _NC = _build_program()
    nc = _NC

    x = np.asarray(x, dtype=np.float32)
    Wq, Wk, Wv, Wo = (np.asarray(w, dtype=np.float32) for w in (Wq, Wk, Wv, Wo))
    am = np.asarray(attn_mask, dtype=np.float32).reshape(L, L)
    causal = np.tril(np.ones((L, L), dtype=bool))
    assert np.array_equal(am, np.where(causal, 0.0, -1e9).astype(np.float32)), (
        "kernel hardcodes a causal additive mask"
    )

    in_maps = _host_inputs(x, Wq, Wk, Wv, Wo, timelike_mask)
    res = run_bass_kernel_spmd(
        nc, in_maps, core_ids=list(range(N_CORES)), trace=_trace
    )
    outp = np.stack(
        [
            sum(res.results[b * HPC + g]["out"] for g in range(HPC))
            for b in range(B)
        ]
    ).astype(np.float32)
    kernel.last_results = res
    return outp


# revision 22
# speedup vs baseline: 1.0623x; 1.0623x over previous
"""LorentzTransformer Trainium2 kernel.

Full inputs in, full output out. Sharding: 8 cores = 2 batches x 4 head
groups (4 heads / 256 channels each). Host pre-transposes x and the weight
shards so every on-chip matmul has its contraction dim on partitions.

Per-core pipeline (all on-chip, fp32 storage):
  QT/KT = W-proj of x (head channels on partitions, seq on free)
  V     = natural-layout proj, augmented with a ones column (softmax denom)
  Qeff  = Q * (1 - 2*alpha*sf*m) / sqrt(dh); sf via PE partition-sum matmuls
  scoresT[k,q] per head -> exp on ACT -> causal via one triangular 0/1 tile
  AV + denom in one PSUM accumulation group; normalize via partition_broadcast
  partial out = A @ Wo_shard.T -> host sums the 4 head-group partials per batch
"""

import numpy as np

from concourse import bacc
import concourse.tile as tile
import concourse.mybir as mybir
from concourse.bass_utils import run_bass_kernel_spmd

B, L, D, H = 2, 1024, 1024, 16
DH = D // H  # 64
ALPHA = 0.25
SCALE = float(np.sqrt(DH))  # 8.0
HPC = 4          # heads per core
DPC = HPC * DH   # 256 channels per core
N_CORES = 8
P = 128
NQC = L // 512   # q chunks of 512
NKT = L // P     # k tiles of 128

FP = mybir.dt.float32
# PE compute dtype: fp16 runs the PE at full rate on the normal datapath
# (the HAM clock gate ignores fp32r matmuls and throttles to 1.2 GHz), gets
# fast-weight-load, and keeps 11 mantissa bits. PSUM accumulation is fp32.
FPC = mybir.dt.float16
NPC = np.float16




def _build_program():
    nc = bacc.Bacc("TRN2", target_bir_lowering=False)

    xT = nc.dram_tensor("xT", [D, L], FPC, kind="ExternalInput")
    wqT = nc.dram_tensor("wqT", [D, DPC], FPC, kind="ExternalInput")
    wkT = nc.dram_tensor("wkT", [D, DPC], FPC, kind="ExternalInput")
    wvT = nc.dram_tensor("wvT", [D, DPC], FPC, kind="ExternalInput")
    woT = nc.dram_tensor("woT", [DPC, D], FPC, kind="ExternalInput")
    normblk = nc.dram_tensor("normblk", [P, 2, 4], FPC, kind="ExternalInput")
    sprime = nc.dram_tensor("sprime", [2, 2, P], FPC, kind="ExternalInput")
    maskT = nc.dram_tensor("maskT", [P, P], FPC, kind="ExternalInput")
    out = nc.dram_tensor("out", [L, D], FP, kind="ExternalOutput")

    with tile.TileContext(nc) as tc:
        with (
            tc.tile_pool(name="persist", bufs=1) as persist,
            tc.tile_pool(name="work", bufs=3) as work,
            tc.tile_pool(name="expp", bufs=6) as expp,
            tc.tile_pool(name="sm", bufs=4) as smp,
            tc.tile_pool(name="ost", bufs=3) as ost,
            tc.tile_pool(name="psA", bufs=2, space="PSUM") as psA,
            tc.tile_pool(name="psS", bufs=3, space="PSUM") as psS,
            tc.tile_pool(name="psV", bufs=3, space="PSUM") as psV,
        ):
            # ---- load inputs ----
            xT_r = xT.rearrange("(o p) l -> p o l", p=P)
            xT_sb = [persist.tile([P, L], FPC, tag=f"xT{k}", name=f"xT{k}") for k in range(D // P)]
            wq_r = wqT.rearrange("(o p) n -> p o n", p=P)
            wq_sb = [persist.tile([P, DPC], FPC, tag=f"wq{k}", name=f"wq{k}") for k in range(D // P)]
            wk_r = wkT.rearrange("(o p) n -> p o n", p=P)
            wk_sb = [persist.tile([P, DPC], FPC, tag=f"wk{k}", name=f"wk{k}") for k in range(D // P)]
            wv_r = wvT.rearrange("(o p) n -> p o n", p=P)
            wv_sb = [persist.tile([P, DPC], FPC, tag=f"wv{k}", name=f"wv{k}") for k in range(D // P)]
            for k in range(D // P):
                nc.sync.dma_start(xT_sb[k][:], xT_r[:, k])
                nc.sync.dma_start(wq_sb[k][:], wq_r[:, k])
            for k in range(D // P):
                nc.sync.dma_start(wk_sb[k][:], wk_r[:, k])
                nc.sync.dma_start(wv_sb[k][:], wv_r[:, k])
            wo_sb = persist.tile([P, DPC // P, D], FPC, tag="wo")
            nc.sync.dma_start(wo_sb[:], woT.rearrange("(o p) n -> p o n", p=P))
            nb_sb = persist.tile([P, 2, 4], FPC, tag="nb")
            nc.sync.dma_start(nb_sb[:], normblk[:])
            sp_sb = persist.tile([2, 2, P], FPC, tag="sp")
            nc.sync.dma_start(sp_sb[:], sprime[:])
            mk_sb = persist.tile([P, P], FPC, tag="mk")
            nc.sync.dma_start(mk_sb[:], maskT[:])

            qT_sb = [persist.tile([P, L], FPC, tag=f"qT{t}", name=f"qT{t}") for t in range(2)]
            kT_sb = [persist.tile([P, L], FPC, tag=f"kT{t}", name=f"kT{t}") for t in range(2)]
            # V' with ones column per (ktile, head)
            v_sb = persist.tile([P, NKT, HPC, DH + 1], FPC, tag="v")
            onecol = persist.tile([P, 1], FP, tag="onecol")
            nc.vector.memset(onecol[:], 1.0)
            nc.vector.tensor_copy(
                v_sb[:, :, :, DH : DH + 1],
                onecol.to_broadcast([P, NKT, HPC, 1]),
            )

            aT_sb = [
                [
                    persist.tile([P, 512], FPC, tag=f"aT{t}_{qc}", name=f"aT{t}_{qc}")
                    for qc in range(NQC)
                ]
                for t in range(2)
            ]

            # ---- QT proj (t-tile at a time) + Lorentz factor, then KT, V ----
            def proj(w_sb, dst, t):
                for qc in range(NQC):
                    ps = psA.tile([P, 512], FP, tag="psA", name="proj")
                    for k in range(D // P):
                        nc.tensor.matmul(
                            ps[:],
                            w_sb[k][:, t * P : (t + 1) * P],
                            xT_sb[k][:, qc * 512 : (qc + 1) * 512],
                            start=(k == 0),
                            stop=(k == D // P - 1),
                        )
                    nc.vector.tensor_copy(dst[t][:, qc * 512 : (qc + 1) * 512], ps[:])

            def lorentz(t):
                # QeffT = QT * (0.125 - 0.0625*sf*m), sf = |Q|/|Qt| per (head,q)
                sq = work.tile([P, L], FPC, tag="sq")
                nc.vector.tensor_mul(sq[:], qT_sb[t][:], qT_sb[t][:])
                sf = work.tile([2, L], FPC, tag="sf")
                for qc in range(NQC):
                    nps = psS.tile([P, 512], FP, tag="psS", name="nps")
                    nc.tensor.matmul(
                        nps[:2, :],
                        nb_sb[:, t, 0:2],
                        sq[:, qc * 512 : (qc + 1) * 512],
                        start=True,
                        stop=True,
                    )
                    nqs = psS.tile([P, 512], FP, tag="psS", name="nqs")
                    nc.tensor.matmul(
                        nqs[:2, :],
                        nb_sb[:, t, 2:4],
                        sq[:, qc * 512 : (qc + 1) * 512],
                        start=True,
                        stop=True,
                    )
                    brcp = smp.tile([2, 512], FP, tag="brcp")
                    nc.vector.reciprocal_approx_fast(brcp[:], nqs[0:2, :])
                    rat = smp.tile([2, 512], FP, tag="rat")
                    nc.vector.tensor_mul(rat[:], nps[0:2, :], brcp[:])
                    nc.scalar.activation(
                        sf[:, qc * 512 : (qc + 1) * 512],
                        rat[:],
                        mybir.ActivationFunctionType.Sqrt,
                    )
                for qc in range(NQC):
                    gps = psS.tile([P, 512], FP, tag="psS", name="gps")
                    nc.tensor.matmul(
                        gps[:],
                        sp_sb[:, t, :],
                        sf[:, qc * 512 : (qc + 1) * 512],
                        start=True,
                        stop=True,
                    )
                    fp_sb = smp.tile([P, 512], FPC, tag="fp")
                    nc.vector.tensor_scalar_add(fp_sb[:], gps[:], 1.0 / SCALE)
                    nc.vector.tensor_mul(
                        qT_sb[t][:, qc * 512 : (qc + 1) * 512],
                        qT_sb[t][:, qc * 512 : (qc + 1) * 512],
                        fp_sb[:],
                    )

            for t in range(2):
                proj(wq_sb, qT_sb, t)
                lorentz(t)
            for t in range(2):
                proj(wk_sb, kT_sb, t)

            # ---- V natural layout: out[l, dv], packed into V' ----
            for lt in range(NKT):
                ps = psA.tile([P, 512], FP, tag="psA", name="vproj")
                for k in range(D // P):
                    nc.tensor.matmul(
                        ps[:, :DPC],
                        xT_sb[k][:, lt * P : (lt + 1) * P],
                        wv_sb[k][:, :],
                        start=(k == 0),
                        stop=(k == D // P - 1),
                    )
                nc.vector.tensor_copy(
                    v_sb[:, lt, :, :DH],
                    ps[:, :DPC].rearrange("p (h d) -> p h d", h=HPC),
                )

            # ---- attention (head pairs row-packed) interleaved with Wo ----
            def attn_group(t, qc):
                avs = [
                    psV.tile([DH + 1, 512], FP, tag="psV", name=f"av{hl}")
                    for hl in range(2)
                ]
                nkt = 4 * qc + 4  # causal: k tiles 0..4qc+3
                for kt in range(nkt):
                    off = max(0, (kt - 4 * qc) * P)  # first visible q col
                    n = 512 - off
                    exs = []
                    for hl in range(2):
                        base = hl * DH
                        sc = psS.tile([P, 512], FP, tag="psS", name=f"sc{hl}")
                        nc.tensor.matmul(
                            sc[:, off:512],
                            kT_sb[t][base : base + DH, kt * P : (kt + 1) * P],
                            qT_sb[t][
                                base : base + DH,
                                qc * 512 + off : (qc + 1) * 512,
                            ],
                            start=True,
                            stop=True,
                            tile_position=(base, 0),
                        )
                        ex = expp.tile([P, 512], FPC, tag="ex", name=f"ex{hl}")
                        nc.scalar.activation(
                            ex[:, off:512],
                            sc[:, off:512],
                            mybir.ActivationFunctionType.Exp,
                        )
                        j = kt - 4 * qc
                        if j >= 0:  # diagonal block gets the triangular mask
                            nc.vector.tensor_mul(
                                ex[:, j * P : (j + 1) * P],
                                ex[:, j * P : (j + 1) * P],
                                mk_sb[:],
                            )
                        exs.append(ex)
                    for hl in range(2):
                        nc.tensor.matmul(
                            avs[hl][:, off:512],
                            v_sb[:, kt, 2 * t + hl, :],
                            exs[hl][:, off:512],
                            start=(kt == 0),
                            stop=(kt == nkt - 1),
                        )
                for hl in range(2):
                    base = hl * DH
                    # free the PSUM bank fast: copy numerator + denom out
                    avr = work.tile([DH, 512], FP, tag="avr", name="avr")
                    nc.vector.tensor_copy(avr[:], avs[hl][:DH, :])
                    den = smp.tile([1, 512], FP, tag="den")
                    nc.scalar.copy(den[:], avs[hl][DH : DH + 1, :])
                    rc = smp.tile([1, 512], FP, tag="rc")
                    nc.vector.reciprocal_approx_fast(rc[:], den[:])
                    bc = smp.tile([DH, 512], FP, tag="bc")
                    nc.gpsimd.partition_broadcast(bc[:], rc[:], channels=DH)
                    nc.vector.tensor_mul(
                        aT_sb[t][qc][base : base + DH, :],
                        avr[:],
                        bc[:],
                    )

            def wo_tile(lt):
                qc = lt // 4
                for jc in range(NQC):
                    ps = psA.tile([P, 512], FP, tag="psA", name="wops")
                    for t2 in range(2):
                        nc.tensor.matmul(
                            ps[:],
                            aT_sb[t2][qc][:, (lt % 4) * P : (lt % 4 + 1) * P],
                            wo_sb[:, t2, jc * 512 : (jc + 1) * 512],
                            start=(t2 == 0),
                            stop=(t2 == 1),
                        )
                    oc = ost.tile([P, 512], FP, tag="oc")
                    nc.vector.tensor_copy(oc[:], ps[:])
                    nc.sync.dma_start(
                        out[lt * P : (lt + 1) * P, jc * 512 : (jc + 1) * 512], oc[:]
                    )

            for qc in range(NQC):
                for t in range(2):
                    attn_group(t, qc)
                for lt in range(4 * qc, 4 * qc + 4):
                    wo_tile(lt)

    nc.compile()
    return nc


_NC = None


def _host_inputs(x, Wq, Wk, Wv, Wo, timelike_mask):
    m_full = np.asarray(timelike_mask).astype(np.float32)
    mt = np.tril(np.ones((P, P), dtype=np.float32)).T.copy()  # maskT[k,q]=1 iff k<=q
    in_maps = []
    for c in range(N_CORES):
        b, g = divmod(c, HPC)
        sl = slice(g * DPC, (g + 1) * DPC)
        m = m_full[sl]  # [256]
        nb = np.zeros((P, 2, 4), dtype=np.float32)
        sp = np.zeros((2, 2, P), dtype=np.float32)
        for t in range(2):
            m_t = m[t * P : (t + 1) * P]
            nb[0:DH, t, 0] = 1.0
            nb[DH:P, t, 1] = 1.0
            nb[0:DH, t, 2] = m_t[0:DH]
            nb[DH:P, t, 3] = m_t[DH:P]
            coef = -2.0 * ALPHA / SCALE  # -0.0625
            sp[0, t, 0:DH] = coef * m_t[0:DH]
            sp[1, t, DH:P] = coef * m_t[DH:P]
        in_maps.append(
            {
                "xT": np.ascontiguousarray(x[b].T).astype(NPC),
                "wqT": np.ascontiguousarray(Wq[sl, :].T).astype(NPC),
                "wkT": np.ascontiguousarray(Wk[sl, :].T).astype(NPC),
                "wvT": np.ascontiguousarray(Wv[sl, :].T).astype(NPC),
                "woT": np.ascontiguousarray(Wo[:, sl].T).astype(NPC),
                "normblk": nb.astype(NPC),
                "sprime": sp.astype(NPC),
                "maskT": mt.astype(NPC),
            }
        )
    return in_maps


def kernel(x, Wq, Wk, Wv, Wo, timelike_mask, attn_mask, _trace=False):
    global _NC
    if _NC is None:
        _NC = _build_program()
    nc = _NC

    x = np.asarray(x, dtype=np.float32)
    Wq, Wk, Wv, Wo = (np.asarray(w, dtype=np.float32) for w in (Wq, Wk, Wv, Wo))
    am = np.asarray(attn_mask, dtype=np.float32).reshape(L, L)
    causal = np.tril(np.ones((L, L), dtype=bool))
    assert np.array_equal(am, np.where(causal, 0.0, -1e9).astype(np.float32)), (
        "kernel hardcodes a causal additive mask"
    )

    in_maps = _host_inputs(x, Wq, Wk, Wv, Wo, timelike_mask)
    res = run_bass_kernel_spmd(
        nc, in_maps, core_ids=list(range(N_CORES)), trace=_trace
    )
    outp = np.stack(
        [
            sum(res.results[b * HPC + g]["out"] for g in range(HPC))
            for b in range(B)
        ]
    ).astype(np.float32)
    kernel.last_results = res
    return outp


# revision 23
# speedup vs baseline: 1.1120x; 1.0469x over previous
"""LorentzTransformer Trainium2 kernel.

Full inputs in, full output out. Sharding: 8 cores = 2 batches x 4 head
groups (4 heads / 256 channels each). Host pre-transposes x and the weight
shards so every on-chip matmul has its contraction dim on partitions.

Per-core pipeline (all on-chip, fp32 storage):
  QT/KT = W-proj of x (head channels on partitions, seq on free)
  V     = natural-layout proj, augmented with a ones column (softmax denom)
  Qeff  = Q * (1 - 2*alpha*sf*m) / sqrt(dh); sf via PE partition-sum matmuls
  scoresT[k,q] per head -> exp on ACT -> causal via one triangular 0/1 tile
  AV + denom in one PSUM accumulation group; normalize via partition_broadcast
  partial out = A @ Wo_shard.T -> host sums the 4 head-group partials per batch
"""

import numpy as np

from concourse import bacc
import concourse.tile as tile
import concourse.mybir as mybir
from concourse.bass_utils import run_bass_kernel_spmd

B, L, D, H = 2, 1024, 1024, 16
DH = D // H  # 64
ALPHA = 0.25
SCALE = float(np.sqrt(DH))  # 8.0
HPC = 4          # heads per core
DPC = HPC * DH   # 256 channels per core
N_CORES = 8
P = 128
NQC = L // 512   # q chunks of 512
NKT = L // P     # k tiles of 128

FP = mybir.dt.float32
# PE compute dtype: fp16 runs the PE at full rate on the normal datapath
# (the HAM clock gate ignores fp32r matmuls and throttles to 1.2 GHz), gets
# fast-weight-load, and keeps 11 mantissa bits. PSUM accumulation is fp32.
FPC = mybir.dt.float16
NPC = np.float16




def _build_program():
    nc = bacc.Bacc("TRN2", target_bir_lowering=False)

    xT = nc.dram_tensor("xT", [D, L], FPC, kind="ExternalInput")
    wqT = nc.dram_tensor("wqT", [D, DPC], FPC, kind="ExternalInput")
    wkT = nc.dram_tensor("wkT", [D, DPC], FPC, kind="ExternalInput")
    wvT = nc.dram_tensor("wvT", [D, DPC], FPC, kind="ExternalInput")
    woT = nc.dram_tensor("woT", [DPC, D], FPC, kind="ExternalInput")
    normblk = nc.dram_tensor("normblk", [P, 2, 4], FPC, kind="ExternalInput")
    sprime = nc.dram_tensor("sprime", [2, 2, P], FPC, kind="ExternalInput")
    maskT = nc.dram_tensor("maskT", [P, P], FPC, kind="ExternalInput")
    out = nc.dram_tensor("out", [L, D], FP, kind="ExternalOutput")

    with tile.TileContext(nc) as tc:
        with (
            tc.tile_pool(name="persist", bufs=1) as persist,
            tc.tile_pool(name="work", bufs=3) as work,
            tc.tile_pool(name="expp", bufs=8) as expp,
            tc.tile_pool(name="sm", bufs=6) as smp,
            tc.tile_pool(name="ost", bufs=3) as ost,
            tc.tile_pool(name="psA", bufs=2, space="PSUM") as psA,
            tc.tile_pool(name="psS", bufs=3, space="PSUM") as psS,
            tc.tile_pool(name="psV", bufs=3, space="PSUM") as psV,
        ):
            # ---- load inputs ----
            xT_r = xT.rearrange("(o p) l -> p o l", p=P)
            xT_sb = [persist.tile([P, L], FPC, tag=f"xT{k}", name=f"xT{k}") for k in range(D // P)]
            wq_r = wqT.rearrange("(o p) n -> p o n", p=P)
            wq_sb = [persist.tile([P, DPC], FPC, tag=f"wq{k}", name=f"wq{k}") for k in range(D // P)]
            wk_r = wkT.rearrange("(o p) n -> p o n", p=P)
            wk_sb = [persist.tile([P, DPC], FPC, tag=f"wk{k}", name=f"wk{k}") for k in range(D // P)]
            wv_r = wvT.rearrange("(o p) n -> p o n", p=P)
            wv_sb = [persist.tile([P, DPC], FPC, tag=f"wv{k}", name=f"wv{k}") for k in range(D // P)]
            for k in range(D // P):
                nc.sync.dma_start(xT_sb[k][:], xT_r[:, k])
                nc.sync.dma_start(wq_sb[k][:], wq_r[:, k])
            for k in range(D // P):
                nc.sync.dma_start(wk_sb[k][:], wk_r[:, k])
                nc.sync.dma_start(wv_sb[k][:], wv_r[:, k])
            wo_sb = persist.tile([P, DPC // P, D], FPC, tag="wo")
            nc.sync.dma_start(wo_sb[:], woT.rearrange("(o p) n -> p o n", p=P))
            nb_sb = persist.tile([P, 2, 4], FPC, tag="nb")
            nc.sync.dma_start(nb_sb[:], normblk[:])
            sp_sb = persist.tile([2, 2, P], FPC, tag="sp")
            nc.sync.dma_start(sp_sb[:], sprime[:])
            mk_sb = persist.tile([P, P], FPC, tag="mk")
            nc.sync.dma_start(mk_sb[:], maskT[:])

            qT_sb = [persist.tile([P, L], FPC, tag=f"qT{t}", name=f"qT{t}") for t in range(2)]
            kT_sb = [persist.tile([P, L], FPC, tag=f"kT{t}", name=f"kT{t}") for t in range(2)]
            # V' with ones column per (ktile, head)
            v_sb = persist.tile([P, NKT, HPC, DH + 1], FPC, tag="v")
            onecol = persist.tile([P, 1], FP, tag="onecol")
            nc.vector.memset(onecol[:], 1.0)
            nc.vector.tensor_copy(
                v_sb[:, :, :, DH : DH + 1],
                onecol.to_broadcast([P, NKT, HPC, 1]),
            )

            aT_sb = [
                [
                    persist.tile([P, 512], FPC, tag=f"aT{t}_{qc}", name=f"aT{t}_{qc}")
                    for qc in range(NQC)
                ]
                for t in range(2)
            ]

            # ---- QT proj (t-tile at a time) + Lorentz factor, then KT, V ----
            def proj(w_sb, dst, t):
                for qc in range(NQC):
                    ps = psA.tile([P, 512], FP, tag="psA", name="proj")
                    for k in range(D // P):
                        nc.tensor.matmul(
                            ps[:],
                            w_sb[k][:, t * P : (t + 1) * P],
                            xT_sb[k][:, qc * 512 : (qc + 1) * 512],
                            start=(k == 0),
                            stop=(k == D // P - 1),
                        )
                    nc.vector.tensor_copy(dst[t][:, qc * 512 : (qc + 1) * 512], ps[:])

            def lorentz(t):
                # QeffT = QT * (0.125 - 0.0625*sf*m), sf = |Q|/|Qt| per (head,q)
                sq = work.tile([P, L], FPC, tag="sq")
                nc.vector.tensor_mul(sq[:], qT_sb[t][:], qT_sb[t][:])
                sf = work.tile([2, L], FPC, tag="sf")
                for qc in range(NQC):
                    nps = psS.tile([P, 512], FP, tag="psS", name="nps")
                    nc.tensor.matmul(
                        nps[:2, :],
                        nb_sb[:, t, 0:2],
                        sq[:, qc * 512 : (qc + 1) * 512],
                        start=True,
                        stop=True,
                    )
                    nqs = psS.tile([P, 512], FP, tag="psS", name="nqs")
                    nc.tensor.matmul(
                        nqs[:2, :],
                        nb_sb[:, t, 2:4],
                        sq[:, qc * 512 : (qc + 1) * 512],
                        start=True,
                        stop=True,
                    )
                    brcp = smp.tile([2, 512], FP, tag="brcp")
                    nc.vector.reciprocal_approx_fast(brcp[:], nqs[0:2, :])
                    rat = smp.tile([2, 512], FP, tag="rat")
                    nc.vector.tensor_mul(rat[:], nps[0:2, :], brcp[:])
                    nc.scalar.activation(
                        sf[:, qc * 512 : (qc + 1) * 512],
                        rat[:],
                        mybir.ActivationFunctionType.Sqrt,
                    )
                for qc in range(NQC):
                    gps = psS.tile([P, 512], FP, tag="psS", name="gps")
                    nc.tensor.matmul(
                        gps[:],
                        sp_sb[:, t, :],
                        sf[:, qc * 512 : (qc + 1) * 512],
                        start=True,
                        stop=True,
                    )
                    fp_sb = smp.tile([P, 512], FPC, tag="fp")
                    nc.vector.tensor_scalar_add(fp_sb[:], gps[:], 1.0 / SCALE)
                    nc.vector.tensor_mul(
                        qT_sb[t][:, qc * 512 : (qc + 1) * 512],
                        qT_sb[t][:, qc * 512 : (qc + 1) * 512],
                        fp_sb[:],
                    )

            for t in range(2):
                proj(wq_sb, qT_sb, t)
                lorentz(t)
            for t in range(2):
                proj(wk_sb, kT_sb, t)

            # ---- V natural layout: out[l, dv], packed into V' ----
            for lt in range(NKT):
                ps = psA.tile([P, 512], FP, tag="psA", name="vproj")
                for k in range(D // P):
                    nc.tensor.matmul(
                        ps[:, :DPC],
                        xT_sb[k][:, lt * P : (lt + 1) * P],
                        wv_sb[k][:, :],
                        start=(k == 0),
                        stop=(k == D // P - 1),
                    )
                nc.vector.tensor_copy(
                    v_sb[:, lt, :, :DH],
                    ps[:, :DPC].rearrange("p (h d) -> p h d", h=HPC),
                )

            # ---- attention (head pairs row-packed) interleaved with Wo ----
            def attn_group(t, qc):
                avs = [
                    psV.tile([DH + 1, 512], FP, tag="psV", name=f"av{hl}")
                    for hl in range(2)
                ]
                nkt = 4 * qc + 4  # causal: k tiles 0..4qc+3
                for kt in range(nkt):
                    off = max(0, (kt - 4 * qc) * P)  # first visible q col
                    n = 512 - off
                    exs = []
                    for hl in range(2):
                        base = hl * DH
                        sc = psS.tile([P, 512], FP, tag="psS", name=f"sc{hl}")
                        nc.tensor.matmul(
                            sc[:, off:512],
                            kT_sb[t][base : base + DH, kt * P : (kt + 1) * P],
                            qT_sb[t][
                                base : base + DH,
                                qc * 512 + off : (qc + 1) * 512,
                            ],
                            start=True,
                            stop=True,
                            tile_position=(base, 0),
                        )
                        ex = expp.tile([P, 512], FPC, tag="ex", name=f"ex{hl}")
                        nc.scalar.activation(
                            ex[:, off:512],
                            sc[:, off:512],
                            mybir.ActivationFunctionType.Exp,
                        )
                        j = kt - 4 * qc
                        if j >= 0:  # diagonal block gets the triangular mask
                            nc.vector.tensor_mul(
                                ex[:, j * P : (j + 1) * P],
                                ex[:, j * P : (j + 1) * P],
                                mk_sb[:],
                            )
                        exs.append(ex)
                    for hl in range(2):
                        nc.tensor.matmul(
                            avs[hl][:, off:512],
                            v_sb[:, kt, 2 * t + hl, :],
                            exs[hl][:, off:512],
                            start=(kt == 0),
                            stop=(kt == nkt - 1),
                        )
                for hl in range(2):
                    base = hl * DH
                    # free the PSUM bank fast: copy numerator + denom out
                    avr = work.tile([DH, 512], FP, tag="avr", name="avr")
                    nc.vector.tensor_copy(avr[:], avs[hl][:DH, :])
                    den = smp.tile([1, 512], FP, tag="den")
                    nc.vector.tensor_copy(den[:], avs[hl][DH : DH + 1, :])
                    rc = smp.tile([1, 512], FP, tag="rc")
                    nc.vector.reciprocal_approx_fast(rc[:], den[:])
                    bc = smp.tile([DH, 512], FP, tag="bc")
                    nc.gpsimd.partition_broadcast(bc[:], rc[:], channels=DH)
                    nc.vector.tensor_mul(
                        aT_sb[t][qc][base : base + DH, :],
                        avr[:],
                        bc[:],
                    )

            def wo_tile(lt):
                qc = lt // 4
                for jc in range(NQC):
                    ps = psA.tile([P, 512], FP, tag="psA", name="wops")
                    for t2 in range(2):
                        nc.tensor.matmul(
                            ps[:],
                            aT_sb[t2][qc][:, (lt % 4) * P : (lt % 4 + 1) * P],
                            wo_sb[:, t2, jc * 512 : (jc + 1) * 512],
                            start=(t2 == 0),
                            stop=(t2 == 1),
                        )
                    oc = ost.tile([P, 512], FP, tag="oc")
                    nc.vector.tensor_copy(oc[:], ps[:])
                    nc.sync.dma_start(
                        out[lt * P : (lt + 1) * P, jc * 512 : (jc + 1) * 512], oc[:]
                    )

            for qc in range(NQC):
                for t in range(2):
                    attn_group(t, qc)
                for lt in range(4 * qc, 4 * qc + 4):
                    wo_tile(lt)

    nc.compile()
    return nc


_NC = None


def _host_inputs(x, Wq, Wk, Wv, Wo, timelike_mask):
    m_full = np.asarray(timelike_mask).astype(np.float32)
    mt = np.tril(np.ones((P, P), dtype=np.float32)).T.copy()  # maskT[k,q]=1 iff k<=q
    in_maps = []
    for c in range(N_CORES):
        b, g = divmod(c, HPC)
        sl = slice(g * DPC, (g + 1) * DPC)
        m = m_full[sl]  # [256]
        nb = np.zeros((P, 2, 4), dtype=np.float32)
        sp = np.zeros((2, 2, P), dtype=np.float32)
        for t in range(2):
            m_t = m[t * P : (t + 1) * P]
            nb[0:DH, t, 0] = 1.0
            nb[DH:P, t, 1] = 1.0
            nb[0:DH, t, 2] = m_t[0:DH]
            nb[DH:P, t, 3] = m_t[DH:P]
            coef = -2.0 * ALPHA / SCALE  # -0.0625
            sp[0, t, 0:DH] = coef * m_t[0:DH]
            sp[1, t, DH:P] = coef * m_t[DH:P]
        in_maps.append(
            {
                "xT": np.ascontiguousarray(x[b].T).astype(NPC),
                "wqT": np.ascontiguousarray(Wq[sl, :].T).astype(NPC),
                "wkT": np.ascontiguousarray(Wk[sl, :].T).astype(NPC),
                "wvT": np.ascontiguousarray(Wv[sl, :].T).astype(NPC),
                "woT": np.ascontiguousarray(Wo[:, sl].T).astype(NPC),
                "normblk": nb.astype(NPC),
                "sprime": sp.astype(NPC),
                "maskT": mt.astype(NPC),
            }
        )
    return in_maps


def kernel(x, Wq, Wk, Wv, Wo, timelike_mask, attn_mask, _trace=False):
    global _NC
    if _NC is None:
        _NC = _build_program()
    nc = _NC

    x = np.asarray(x, dtype=np.float32)
    Wq, Wk, Wv, Wo = (np.asarray(w, dtype=np.float32) for w in (Wq, Wk, Wv, Wo))
    am = np.asarray(attn_mask, dtype=np.float32).reshape(L, L)
    causal = np.tril(np.ones((L, L), dtype=bool))
    assert np.array_equal(am, np.where(causal, 0.0, -1e9).astype(np.float32)), (
        "kernel hardcodes a causal additive mask"
    )

    in_maps = _host_inputs(x, Wq, Wk, Wv, Wo, timelike_mask)
    res = run_bass_kernel_spmd(
        nc, in_maps, core_ids=list(range(N_CORES)), trace=_trace
    )
    outp = np.stack(
        [
            sum(res.results[b * HPC + g]["out"] for g in range(HPC))
            for b in range(B)
        ]
    ).astype(np.float32)
    kernel.last_results = res
    return outp
